# revision 27
# baseline (speedup 1.0000x reference)
"""Trainium2 Bass kernel for nn_Decoder (LSTM decoder with residual output feedback).

Model (per batch row):
    h0 = c0 = z @ W_proj.T + b_proj                      # [B, H]
    y0 = x[:, -1, :]                                     # [B, X]
    per step t: gates = y_{t-1} @ W_ih.T + h_{t-1} @ W_hh.T + (b_ih + b_hh)
                i, f, g, o = split(gates); c = sig(f)*c + sig(i)*tanh(g)
                h = sig(o)*tanh(c); y_t = y_{t-1} + h @ W_out.T + b_out
    out = stack(y_1..y_T)                                # [B, T, Y]

Strategy:
  * Pure data-parallel over batch: B=1024 -> 128 rows/core on 8 NeuronCores,
    weights replicated, zero collectives; outputs concatenated on the host.
  * All state is kept TRANSPOSED on chip ([feature, batch], batch on the free
    axis): gates come out of the PE array as gates^T with the weights as the
    stationary operand, and the elementwise state update directly produces
    h^T, which feeds the next step's matmuls as the moving operand -- the
    recurrence contains no transposes at all.
  * Each core's 128 rows are further split into two independent 64-row halves
    executed half-a-step out of phase: while one half runs its sigmod/tanh +
    c/h update chain on the Scalar/Vector engines, the other half's matmul
    burst keeps the TensorEngine busy (and the PE clock un-throttled).
  * Matmuls run in bf16 (f32 PSUM accumulation); c and y state stay f32.
  * Gate rows are pre-permuted on the host gate-major (f,g | i,o) into two
    one-bank PSUM tiles: the f/g bank closes ~1.1us before each burst ends
    (so sig(f)/tanh(g)/f*c run hidden under the matmuls) and the i/o bank
    closes last, leaving only sig(i)->i*g->c+=->tanh(c)->o*tanh(c) on the
    recurrence-critical tail (all bf16, DVE 2x mode).
  * y_t accumulates in a persistent PSUM bank (y_t = y0 + psum); the output
    is staged transposed in SBUF and un-transposed on the host.
"""



import os
from contextlib import ExitStack

import ml_dtypes
import numpy as np

import concourse.bass as bass
import concourse.tile as tile
from concourse import bacc, mybir
from concourse.bass_utils import run_bass_kernel_spmd


F32 = mybir.dt.float32
BF16 = mybir.dt.bfloat16
SIG = mybir.ActivationFunctionType.Sigmoid
TANH = mybir.ActivationFunctionType.Tanh

B_TOT = 1024
N_CORES = 8
B = 128          # rows per core
BH = 64          # rows per half
ZD, XD, YD, H = 128, 64, 64, 512
HC, GC = 4, 16

LAST_RESULTS = None
_BUILD_CACHE = {}



def _gate_row_order():
    """Permuted gate-row order: gate-major blocks (f_0..f_3, g_0..g_3, i_0..i_3, o_0..o_3).

    Original gate layout along 4H: i=[0,512), f=[512,1024), g=[1024,1536), o=[1536,2048).
    Gate-major ordering makes every activation/elementwise op a CONTIGUOUS
    [128, 256] slice of the gate PSUM. The f/g gates go to PSUM bank A
    (closed ~1.1us before the burst ends, so sig(f)/tanh(g) run mid-burst)
    and i/o to bank B (whose residual chain after the burst is shortest:
    sig(i) -> i*g -> c+= -> tanh(c) -> o*tanh(c)).
    """
    idx = []
    for base in (512, 1024, 0, 1536):  # f, g, i, o
        for k in range(HC):
            idx.extend(range(base + k * 128, base + (k + 1) * 128))
    return np.asarray(idx)



def _prep_consts(W_ih, W_hh, b_ih, b_hh, W_proj, b_proj, W_out, b_out):
    bf = ml_dtypes.bfloat16
    order = _gate_row_order()
    Wg = W_hh[order]                       # [2048, 512] permuted rows
    Wi = W_ih[order]                       # [2048, 64]
    bt = (b_ih + b_hh)[order]              # [2048]

    wg_h = np.empty((128, HC * GC * 128), dtype=bf)
    for k in range(HC):
        for s in range(GC):
            blk = Wg[s * 128:(s + 1) * 128, k * 128:(k + 1) * 128].T  # [K,M]
            wg_h[:, (k * GC + s) * 128:(k * GC + s + 1) * 128] = blk.astype(bf)

    wg_y = np.empty((YD + 1, GC * 128), dtype=bf)
    for s in range(GC):
        wg_y[0:YD, s * 128:(s + 1) * 128] = Wi[s * 128:(s + 1) * 128, :].T.astype(bf)
        wg_y[YD, s * 128:(s + 1) * 128] = bt[s * 128:(s + 1) * 128].astype(bf)

    wout = np.zeros((128, HC * 128), dtype=bf)
    for k in range(HC):
        wout[:, k * 128:k * 128 + YD] = W_out[:, k * 128:(k + 1) * 128].T.astype(bf)

    wproj = np.empty((ZD, H), dtype=bf)
    for m in range(HC):
        wproj[:, m * 128:(m + 1) * 128] = W_proj[m * 128:(m + 1) * 128, :].T.astype(bf)

    bprojT = b_proj.reshape(HC, 128).T.copy().astype(np.float32)  # [128, HC]
    bout1 = b_out.reshape(1, YD).astype(bf)
    ones1 = np.ones((1, B), dtype=bf)
    return dict(wg_h=wg_h, wg_y=wg_y, wout=wout, wproj=wproj,
                bprojT=bprojT, bout1=bout1, ones1=ones1)



def _j4(ap, c):
    return ap.rearrange("p (j c) -> p j c", c=c)


def _build(T):
    nc = bacc.Bacc("TRN2", target_bir_lowering=False, debug=False)

    d_zT = nc.dram_tensor("zT", [ZD, B], BF16, kind="ExternalInput")
    d_y0T = nc.dram_tensor("y0T", [YD, B], F32, kind="ExternalInput")
    d_wg_h = nc.dram_tensor("wg_h", [128, HC * GC * 128], BF16, kind="ExternalInput")
    d_wg_y = nc.dram_tensor("wg_y", [YD + 1, GC * 128], BF16, kind="ExternalInput")
    d_wout = nc.dram_tensor("wout", [128, HC * 128], BF16, kind="ExternalInput")
    d_bout1 = nc.dram_tensor("bout1", [1, YD], BF16, kind="ExternalInput")
    d_ones1 = nc.dram_tensor("ones1", [1, B], BF16, kind="ExternalInput")
    d_wproj = nc.dram_tensor("wproj", [ZD, H], BF16, kind="ExternalInput")
    d_bprojT = nc.dram_tensor("bprojT", [128, HC], F32, kind="ExternalInput")
    d_bscan = nc.dram_tensor("bscan", [YD, T], F32, kind="ExternalInput")
    d_out = nc.dram_tensor("out", [YD, T * B], F32, kind="ExternalOutput")

    with ExitStack() as ctx:
        tc = ctx.enter_context(tile.TileContext(nc))
        const = ctx.enter_context(tc.tile_pool(name="const", bufs=1))
        state = ctx.enter_context(tc.tile_pool(name="state", bufs=1))
        actp = ctx.enter_context(tc.tile_pool(name="actp", bufs=4))
        gpsum = ctx.enter_context(tc.tile_pool(name="gpsum", bufs=3, space="PSUM"))
        ypsum = ctx.enter_context(tc.tile_pool(name="ypsum", bufs=1, space="PSUM"))

        wg_h = const.tile([128, HC * GC * 128], BF16)
        wg_y = const.tile([YD + 1, GC * 128], BF16)
        wout = const.tile([128, HC * 128], BF16)
        bout1 = const.tile([1, YD], BF16)
        ones1 = const.tile([1, B], BF16)
        wproj = const.tile([ZD, H], BF16)
        bprojT = const.tile([128, HC], F32)
        bscan = const.tile([YD, T], F32)
        zT = const.tile([ZD, B], BF16)
        y0T = const.tile([YD, B], F32)
        # small init-critical tensors first so h0/c0 compute (and the
        # one-time ACT table load) overlap the big wg_h weight DMA
        for sb, dr in (
            (zT, d_zT), (wproj, d_wproj), (bprojT, d_bprojT), (y0T, d_y0T),
            (ones1, d_ones1), (bout1, d_bout1), (bscan, d_bscan),
            (wg_y, d_wg_y), (wout, d_wout), (wg_h, d_wg_h),
        ):
            nc.sync.dma_start(sb[:, :], dr[:, :])

        # dummy activation to pull the ~2.7us ACT spline-table load off the
        # first decode step's critical path
        warmp = state.tile([1, YD], F32, name="actwarm")
        nc.scalar.activation(warmp[:, :], bout1[:, :], SIG)

        # per-half state; layout [128, 4*64]: H-chunk k at cols k*64
        cT = [state.tile([128, 256], BF16, name=f"cT{h}") for h in range(2)]
        hT = [[state.tile([128, 256], BF16, name=f"hT{p}_{h}") for h in range(2)]
              for p in range(2)]
        yTa = [[state.tile([YD + 1, BH], BF16, name=f"yTa{p}_{h}") for h in range(2)]
               for p in range(2)]
        ysbT = state.tile([YD, T * B], F32)   # out[y, t*128 + h*64 + b]
        yp = [ypsum.tile([128, BH], F32, name=f"yp{h}", tag=f"yp{h}")
              for h in range(2)]

        # --- init (both halves) ---
        for h in range(2):
            bsl = slice(h * BH, (h + 1) * BH)
            h0p = gpsum.tile([128, 512], F32, tag="gA", name=f"h0p{h}")
            for m in range(HC):
                nc.tensor.matmul(
                    h0p[:, m * 64:(m + 1) * 64],
                    lhsT=wproj[:, m * 128:(m + 1) * 128],
                    rhs=zT[:, bsl], start=True, stop=True,
                )
            for m in range(HC):
                nc.vector.tensor_scalar_add(
                    cT[h][:, m * 64:(m + 1) * 64],
                    h0p[:, m * 64:(m + 1) * 64],
                    bprojT[:, m:m + 1],
                )
            nc.vector.tensor_copy(hT[1][h][:, :], cT[h][:, :])
            nc.vector.tensor_copy(yTa[1][h][0:YD, :], y0T[:, bsl])
            nc.vector.memset(yTa[0][h][YD:YD + 1, :], 1.0)
            nc.vector.memset(yTa[1][h][YD:YD + 1, :], 1.0)

        DMA_CHUNK = 32

        def emit_half(t, h):
            pv = (t + 1) % 2
            cu = t % 2
            bsl = slice(h * BH, (h + 1) * BH)
            # two one-bank PSUM tiles: gpA = f,g gates, gpB = i,o gates.
            # Separate tiles -> dependency tracking frees bank A for the
            # activation chain while bank B is still accumulating.
            gpA = gpsum.tile([128, 512], F32, tag="gA", name=f"gpA{t}_{h}")
            gpB = gpsum.tile([128, 512], F32, tag="gB", name=f"gpB{t}_{h}")

            if t > 0:
                for k in range(HC):
                    nc.tensor.matmul(
                        yp[h][:, :],
                        lhsT=wout[:, k * 128:(k + 1) * 128],
                        rhs=hT[pv][h][:, k * 64:(k + 1) * 64],
                        start=(t == 1 and k == 0), stop=False,
                        skip_group_check=True,
                    )
                tp = t - 1
                # y_tp = y0 + psum + (tp+1)*b_out; the bias ramp comes from a
                # host-precomputed table so no bias matmul sits on this path.
                # The yTa write feeds THIS burst's wg_y matmuls below, so it
                # must be emitted before them.
                nc.vector.scalar_tensor_tensor(
                    yTa[tp % 2][h][0:YD, :], yp[h][0:YD, :], bscan[:, tp:tp + 1],
                    y0T[:, bsl], op0=mybir.AluOpType.add, op1=mybir.AluOpType.add)
                nc.vector.scalar_tensor_tensor(
                    ysbT[:, tp * B + h * BH:tp * B + (h + 1) * BH],
                    yp[h][0:YD, :], bscan[:, tp:tp + 1],
                    y0T[:, bsl], op0=mybir.AluOpType.add, op1=mybir.AluOpType.add)
                if h == 1 and (tp % DMA_CHUNK == DMA_CHUNK - 1):
                    lo = (tp // DMA_CHUNK) * DMA_CHUNK * B
                    nc.sync.dma_start(d_out[:, lo:(tp + 1) * B],
                                      ysbT[:, lo:(tp + 1) * B])

            def gates_for(gp, s0, ns):
                for s in range(s0, s0 + ns):
                    for k in range(HC):
                        nc.tensor.matmul(
                            gp[:, (s - s0) * 64:(s - s0 + 1) * 64],
                            lhsT=wg_h[:, (k * GC + s) * 128:(k * GC + s + 1) * 128],
                            rhs=hT[pv][h][:, k * 64:(k + 1) * 64],
                            start=(s == s0 and k == 0), stop=False,
                            skip_group_check=True,
                        )
                for s in range(s0, s0 + ns):
                    nc.tensor.matmul(
                        gp[:, (s - s0) * 64:(s - s0 + 1) * 64],
                        lhsT=wg_y[:, s * 128:(s + 1) * 128],
                        rhs=yTa[pv][h][:, :],
                        start=False, stop=True, skip_group_check=True,
                    )

            gates_for(gpA, 0, 8)   # f, g gates: bank A closes ~1.1us before burst end
            gates_for(gpB, 8, 8)   # i, o gates: bank B closes at burst end

            # chain: sig(f)/tanh(g)/f*c run mid-burst off bank A; the tail
            # after the burst is only sig(i) -> i*g -> c+= -> tanh(c) -> h,
            # with sig(o) filling the ACT engine's idle slot between sig(i)
            # and tanh(c) so the greedy scheduler doesn't park a slack op of
            # the next half-step there (that costs ~0.5us on the PE).
            # bf16 intermediates put every tail mul in the DVE 2x perf mode.
            sgf = actp.tile([128, 256], BF16, tag=f"sgf{h}", name=f"sgf{t}_{h}")
            tg = actp.tile([128, 256], BF16, tag=f"tg{h}", name=f"tg{t}_{h}")
            sgi = actp.tile([128, 256], BF16, tag=f"sgi{h}", name=f"sgi{t}_{h}")
            sgo = actp.tile([128, 256], BF16, tag=f"sgo{h}", name=f"sgo{t}_{h}")
            t2 = actp.tile([128, 256], BF16, tag=f"t2{h}", name=f"t2_{t}_{h}")
            t1 = actp.tile([128, 256], BF16, tag=f"t1{h}", name=f"t1_{t}_{h}")
            nc.scalar.activation(sgf[:, :], gpA[:, 0:256], SIG)
            nc.vector.tensor_mul(t2[:, :], sgf[:, :], cT[h][:, :])
            nc.scalar.activation(tg[:, :], gpA[:, 256:512], TANH)
            tch = actp.tile([128, 256], BF16, tag=f"tc{h}", name=f"tc{t}_{h}")
            with tc.high_priority():
                nc.scalar.activation(sgi[:, :], gpB[:, 0:256], SIG)
                nc.scalar.activation(sgo[:, :], gpB[:, 256:512], SIG)
                nc.vector.tensor_mul(t1[:, :], sgi[:, :], tg[:, :])
                nc.vector.tensor_add(cT[h][:, :], t2[:, :], t1[:, :])
                nc.scalar.activation(tch[:, :], cT[h][:, :], TANH)
                nc.vector.tensor_mul(hT[cu][h][:, :], sgo[:, :], tch[:, :])

        for t in range(T):
            emit_half(t, 0)
            emit_half(t, 1)

        # final y tails
        for h in range(2):
            bsl = slice(h * BH, (h + 1) * BH)
            for k in range(HC):
                nc.tensor.matmul(
                    yp[h][:, :],
                    lhsT=wout[:, k * 128:(k + 1) * 128],
                    rhs=hT[(T - 1) % 2][h][:, k * 64:(k + 1) * 64],
                    start=False, stop=(k == HC - 1), skip_group_check=True,
                )
            tp = T - 1
            sl = ysbT[:, tp * B + h * BH:tp * B + (h + 1) * BH]
            nc.vector.scalar_tensor_tensor(
                sl, yp[h][0:YD, :], bscan[:, tp:tp + 1],
                y0T[:, bsl], op0=mybir.AluOpType.add, op1=mybir.AluOpType.add)
        lo = ((T - 1) // DMA_CHUNK) * DMA_CHUNK * B
        nc.sync.dma_start(d_out[:, lo:T * B], ysbT[:, lo:T * B])

    nc.compile()
    return nc


def kernel(z, x, W_ih, W_hh, b_ih, b_hh, W_proj, b_proj, W_out, b_out, y_pred_len):
    global LAST_RESULTS
    z = np.asarray(z, dtype=np.float32)
    x = np.asarray(x, dtype=np.float32)
    T = int(np.asarray(y_pred_len))

    consts = _prep_consts(
        np.asarray(W_ih, np.float32), np.asarray(W_hh, np.float32),
        np.asarray(b_ih, np.float32), np.asarray(b_hh, np.float32),
        np.asarray(W_proj, np.float32), np.asarray(b_proj, np.float32),
        np.asarray(W_out, np.float32), np.asarray(b_out, np.float32),
    )

    if T not in _BUILD_CACHE:
        _BUILD_CACHE[T] = _build(T)
    nc = _BUILD_CACHE[T]
    consts["bscan"] = np.ascontiguousarray(
        np.outer(np.asarray(b_out, np.float32),
                 np.arange(1, T + 1, dtype=np.float32)))

    bf = ml_dtypes.bfloat16
    in_maps = []
    for i in range(N_CORES):
        sl = slice(i * B, (i + 1) * B)
        m = dict(consts)
        m["zT"] = np.ascontiguousarray(z[sl].T.astype(bf))
        m["y0T"] = np.ascontiguousarray(x[sl, -1, :].T.astype(np.float32))
        in_maps.append(m)

    trace = bool(int(os.environ.get("BASS_KERNEL_TRACE", "0")))
    res = run_bass_kernel_spmd(
        nc, in_maps, core_ids=list(range(N_CORES)), trace=trace,
    )
    LAST_RESULTS = res

    outs = [np.ascontiguousarray(
                np.asarray(res.results[i]["out"]).reshape(YD, T, B).transpose(2, 1, 0))
            for i in range(N_CORES)]
    return np.concatenate(outs, axis=0)



# revision 28
# speedup vs baseline: 1.0002x; 1.0002x over previous
"""Trainium2 Bass kernel for nn_Decoder (LSTM decoder with residual output feedback).

Model (per batch row):
    h0 = c0 = z @ W_proj.T + b_proj                      # [B, H]
    y0 = x[:, -1, :]                                     # [B, X]
    per step t: gates = y_{t-1} @ W_ih.T + h_{t-1} @ W_hh.T + (b_ih + b_hh)
                i, f, g, o = split(gates); c = sig(f)*c + sig(i)*tanh(g)
                h = sig(o)*tanh(c); y_t = y_{t-1} + h @ W_out.T + b_out
    out = stack(y_1..y_T)                                # [B, T, Y]

Strategy:
  * Pure data-parallel over batch: B=1024 -> 128 rows/core on 8 NeuronCores,
    weights replicated, zero collectives; outputs concatenated on the host.
  * All state is kept TRANSPOSED on chip ([feature, batch], batch on the free
    axis): gates come out of the PE array as gates^T with the weights as the
    stationary operand, and the elementwise state update directly produces
    h^T, which feeds the next step's matmuls as the moving operand -- the
    recurrence contains no transposes at all.
  * Each core's 128 rows are further split into two independent 64-row halves
    executed half-a-step out of phase: while one half runs its sigmod/tanh +
    c/h update chain on the Scalar/Vector engines, the other half's matmul
    burst keeps the TensorEngine busy (and the PE clock un-throttled).
  * Matmuls run in bf16 (f32 PSUM accumulation); c and y state stay f32.
  * Gate rows are pre-permuted on the host gate-major (f,g | i,o) into two
    one-bank PSUM tiles: the f/g bank closes ~1.1us before each burst ends
    (so sig(f)/tanh(g)/f*c run hidden under the matmuls) and the i/o bank
    closes last, leaving only sig(i)->i*g->c+=->tanh(c)->o*tanh(c) on the
    recurrence-critical tail (all bf16, DVE 2x mode).
  * y_t accumulates in a persistent PSUM bank (y_t = y0 + psum); the output
    is staged transposed in SBUF and un-transposed on the host.
"""



import os
from contextlib import ExitStack

import ml_dtypes
import numpy as np

import concourse.bass as bass
import concourse.tile as tile
from concourse import bacc, mybir
from concourse.bass_utils import run_bass_kernel_spmd


F32 = mybir.dt.float32
BF16 = mybir.dt.bfloat16
SIG = mybir.ActivationFunctionType.Sigmoid
TANH = mybir.ActivationFunctionType.Tanh

B_TOT = 1024
N_CORES = 8
B = 128          # rows per core
BH = 64          # rows per half
ZD, XD, YD, H = 128, 64, 64, 512
HC, GC = 4, 16

LAST_RESULTS = None
_BUILD_CACHE = {}



def _gate_row_order():
    """Permuted gate-row order: gate-major blocks (f_0..f_3, g_0..g_3, i_0..i_3, o_0..o_3).

    Original gate layout along 4H: i=[0,512), f=[512,1024), g=[1024,1536), o=[1536,2048).
    Gate-major ordering makes every activation/elementwise op a CONTIGUOUS
    [128, 256] slice of the gate PSUM. The f/g gates go to PSUM bank A
    (closed ~1.1us before the burst ends, so sig(f)/tanh(g) run mid-burst)
    and i/o to bank B (whose residual chain after the burst is shortest:
    sig(i) -> i*g -> c+= -> tanh(c) -> o*tanh(c)).
    """
    idx = []
    for base in (512, 1024, 0, 1536):  # f, g, i, o
        for k in range(HC):
            idx.extend(range(base + k * 128, base + (k + 1) * 128))
    return np.asarray(idx)



def _prep_consts(W_ih, W_hh, b_ih, b_hh, W_proj, b_proj, W_out, b_out):
    bf = ml_dtypes.bfloat16
    order = _gate_row_order()
    Wg = W_hh[order]                       # [2048, 512] permuted rows
    Wi = W_ih[order]                       # [2048, 64]
    bt = (b_ih + b_hh)[order]              # [2048]

    wg_h = np.empty((128, HC * GC * 128), dtype=bf)
    for k in range(HC):
        for s in range(GC):
            blk = Wg[s * 128:(s + 1) * 128, k * 128:(k + 1) * 128].T  # [K,M]
            wg_h[:, (k * GC + s) * 128:(k * GC + s + 1) * 128] = blk.astype(bf)

    wg_y = np.empty((YD + 1, GC * 128), dtype=bf)
    for s in range(GC):
        wg_y[0:YD, s * 128:(s + 1) * 128] = Wi[s * 128:(s + 1) * 128, :].T.astype(bf)
        wg_y[YD, s * 128:(s + 1) * 128] = bt[s * 128:(s + 1) * 128].astype(bf)

    wout = np.zeros((128, HC * 128), dtype=bf)
    for k in range(HC):
        wout[:, k * 128:k * 128 + YD] = W_out[:, k * 128:(k + 1) * 128].T.astype(bf)

    wproj = np.empty((ZD, H), dtype=bf)
    for m in range(HC):
        wproj[:, m * 128:(m + 1) * 128] = W_proj[m * 128:(m + 1) * 128, :].T.astype(bf)

    bprojT = b_proj.reshape(HC, 128).T.copy().astype(np.float32)  # [128, HC]
    bout1 = b_out.reshape(1, YD).astype(bf)
    ones1 = np.ones((1, B), dtype=bf)
    return dict(wg_h=wg_h, wg_y=wg_y, wout=wout, wproj=wproj,
                bprojT=bprojT, bout1=bout1, ones1=ones1)



def _j4(ap, c):
    return ap.rearrange("p (j c) -> p j c", c=c)


def _build(T):
    nc = bacc.Bacc("TRN2", target_bir_lowering=False, debug=False)

    d_zT = nc.dram_tensor("zT", [ZD, B], BF16, kind="ExternalInput")
    d_y0T = nc.dram_tensor("y0T", [YD, B], F32, kind="ExternalInput")
    d_wg_h = nc.dram_tensor("wg_h", [128, HC * GC * 128], BF16, kind="ExternalInput")
    d_wg_y = nc.dram_tensor("wg_y", [YD + 1, GC * 128], BF16, kind="ExternalInput")
    d_wout = nc.dram_tensor("wout", [128, HC * 128], BF16, kind="ExternalInput")
    d_bout1 = nc.dram_tensor("bout1", [1, YD], BF16, kind="ExternalInput")
    d_ones1 = nc.dram_tensor("ones1", [1, B], BF16, kind="ExternalInput")
    d_wproj = nc.dram_tensor("wproj", [ZD, H], BF16, kind="ExternalInput")
    d_bprojT = nc.dram_tensor("bprojT", [128, HC], F32, kind="ExternalInput")
    d_bscan = nc.dram_tensor("bscan", [YD, T], F32, kind="ExternalInput")
    d_out = nc.dram_tensor("out", [YD, T * B], F32, kind="ExternalOutput")

    with ExitStack() as ctx:
        tc = ctx.enter_context(tile.TileContext(nc))
        const = ctx.enter_context(tc.tile_pool(name="const", bufs=1))
        state = ctx.enter_context(tc.tile_pool(name="state", bufs=1))
        actp = ctx.enter_context(tc.tile_pool(name="actp", bufs=4))
        gpsum = ctx.enter_context(tc.tile_pool(name="gpsum", bufs=4, space="PSUM"))
        gpsumB = ctx.enter_context(tc.tile_pool(name="gpsumB", bufs=2, space="PSUM"))
        ypsum = ctx.enter_context(tc.tile_pool(name="ypsum", bufs=1, space="PSUM"))

        wg_h = const.tile([128, HC * GC * 128], BF16)
        wg_y = const.tile([YD + 1, GC * 128], BF16)
        wout = const.tile([128, HC * 128], BF16)
        bout1 = const.tile([1, YD], BF16)
        ones1 = const.tile([1, B], BF16)
        wproj = const.tile([ZD, H], BF16)
        bprojT = const.tile([128, HC], F32)
        bscan = const.tile([YD, T], F32)
        zT = const.tile([ZD, B], BF16)
        y0T = const.tile([YD, B], F32)
        # small init-critical tensors first so h0/c0 compute (and the
        # one-time ACT table load) overlap the big wg_h weight DMA
        for sb, dr in (
            (zT, d_zT), (wproj, d_wproj), (bprojT, d_bprojT), (y0T, d_y0T),
            (ones1, d_ones1), (bout1, d_bout1), (bscan, d_bscan),
            (wg_y, d_wg_y), (wout, d_wout), (wg_h, d_wg_h),
        ):
            nc.sync.dma_start(sb[:, :], dr[:, :])

        # dummy activation to pull the ~2.7us ACT spline-table load off the
        # first decode step's critical path
        warmp = state.tile([1, YD], F32, name="actwarm")
        nc.scalar.activation(warmp[:, :], bout1[:, :], SIG)

        # per-half state; layout [128, 4*64]: H-chunk k at cols k*64
        cT = [state.tile([128, 256], BF16, name=f"cT{h}") for h in range(2)]
        hT = [[state.tile([128, 256], BF16, name=f"hT{p}_{h}") for h in range(2)]
              for p in range(2)]
        yTa = [[state.tile([YD + 1, BH], BF16, name=f"yTa{p}_{h}") for h in range(2)]
               for p in range(2)]
        ysbT = state.tile([YD, T * B], F32)   # out[y, t*128 + h*64 + b]
        yp = [ypsum.tile([128, BH], F32, name=f"yp{h}", tag=f"yp{h}")
              for h in range(2)]

        # --- init (both halves) ---
        for h in range(2):
            bsl = slice(h * BH, (h + 1) * BH)
            h0p = gpsum.tile([128, 512], F32, tag="gA", name=f"h0p{h}")
            for m in range(HC):
                nc.tensor.matmul(
                    h0p[:, m * 64:(m + 1) * 64],
                    lhsT=wproj[:, m * 128:(m + 1) * 128],
                    rhs=zT[:, bsl], start=True, stop=True,
                )
            for m in range(HC):
                nc.vector.tensor_scalar_add(
                    cT[h][:, m * 64:(m + 1) * 64],
                    h0p[:, m * 64:(m + 1) * 64],
                    bprojT[:, m:m + 1],
                )
            nc.vector.tensor_copy(hT[1][h][:, :], cT[h][:, :])
            nc.vector.tensor_copy(yTa[1][h][0:YD, :], y0T[:, bsl])
            nc.vector.memset(yTa[0][h][YD:YD + 1, :], 1.0)
            nc.vector.memset(yTa[1][h][YD:YD + 1, :], 1.0)

        DMA_CHUNK = 32

        def emit_half(t, h):
            pv = (t + 1) % 2
            cu = t % 2
            bsl = slice(h * BH, (h + 1) * BH)
            # two one-bank PSUM tiles: gpA = f,g gates, gpB = i,o gates.
            # Separate tiles -> dependency tracking frees bank A for the
            # activation chain while bank B is still accumulating.
            gpA = gpsum.tile([128, 512], F32, tag="gA", name=f"gpA{t}_{h}")
            gpB = gpsumB.tile([128, 512], F32, tag="gB", name=f"gpB{t}_{h}")

            if t > 0:
                for k in range(HC):
                    nc.tensor.matmul(
                        yp[h][:, :],
                        lhsT=wout[:, k * 128:(k + 1) * 128],
                        rhs=hT[pv][h][:, k * 64:(k + 1) * 64],
                        start=(t == 1 and k == 0), stop=False,
                        skip_group_check=True,
                    )
                tp = t - 1
                # y_tp = y0 + psum + (tp+1)*b_out; the bias ramp comes from a
                # host-precomputed table so no bias matmul sits on this path.
                # The yTa write feeds THIS burst's wg_y matmuls below, so it
                # must be emitted before them.
                nc.vector.scalar_tensor_tensor(
                    yTa[tp % 2][h][0:YD, :], yp[h][0:YD, :], bscan[:, tp:tp + 1],
                    y0T[:, bsl], op0=mybir.AluOpType.add, op1=mybir.AluOpType.add)
                nc.vector.scalar_tensor_tensor(
                    ysbT[:, tp * B + h * BH:tp * B + (h + 1) * BH],
                    yp[h][0:YD, :], bscan[:, tp:tp + 1],
                    y0T[:, bsl], op0=mybir.AluOpType.add, op1=mybir.AluOpType.add)
                if h == 1 and (tp % DMA_CHUNK == DMA_CHUNK - 1):
                    lo = (tp // DMA_CHUNK) * DMA_CHUNK * B
                    nc.sync.dma_start(d_out[:, lo:(tp + 1) * B],
                                      ysbT[:, lo:(tp + 1) * B])

            def gates_for(gp, s0, ns):
                for s in range(s0, s0 + ns):
                    for k in range(HC):
                        nc.tensor.matmul(
                            gp[:, (s - s0) * 64:(s - s0 + 1) * 64],
                            lhsT=wg_h[:, (k * GC + s) * 128:(k * GC + s + 1) * 128],
                            rhs=hT[pv][h][:, k * 64:(k + 1) * 64],
                            start=(s == s0 and k == 0), stop=False,
                            skip_group_check=True,
                        )
                for s in range(s0, s0 + ns):
                    nc.tensor.matmul(
                        gp[:, (s - s0) * 64:(s - s0 + 1) * 64],
                        lhsT=wg_y[:, s * 128:(s + 1) * 128],
                        rhs=yTa[pv][h][:, :],
                        start=False, stop=True, skip_group_check=True,
                    )

            gates_for(gpA, 0, 8)   # f, g gates: bank A closes ~1.1us before burst end
            gates_for(gpB, 8, 8)   # i, o gates: bank B closes at burst end

            # chain: sig(f)/tanh(g)/f*c run mid-burst off bank A; the tail
            # after the burst is only sig(i) -> i*g -> c+= -> tanh(c) -> h,
            # with sig(o) filling the ACT engine's idle slot between sig(i)
            # and tanh(c) so the greedy scheduler doesn't park a slack op of
            # the next half-step there (that costs ~0.5us on the PE).
            # bf16 intermediates put every tail mul in the DVE 2x perf mode.
            sgf = actp.tile([128, 256], BF16, tag=f"sgf{h}", name=f"sgf{t}_{h}")
            tg = actp.tile([128, 256], BF16, tag=f"tg{h}", name=f"tg{t}_{h}")
            sgi = actp.tile([128, 256], BF16, tag=f"sgi{h}", name=f"sgi{t}_{h}")
            sgo = actp.tile([128, 256], BF16, tag=f"sgo{h}", name=f"sgo{t}_{h}")
            t2 = actp.tile([128, 256], BF16, tag=f"t2{h}", name=f"t2_{t}_{h}")
            t1 = actp.tile([128, 256], BF16, tag=f"t1{h}", name=f"t1_{t}_{h}")
            nc.scalar.activation(sgf[:, :], gpA[:, 0:256], SIG)
            nc.vector.tensor_mul(t2[:, :], sgf[:, :], cT[h][:, :])
            nc.scalar.activation(tg[:, :], gpA[:, 256:512], TANH)
            tch = actp.tile([128, 256], BF16, tag=f"tc{h}", name=f"tc{t}_{h}")
            with tc.high_priority():
                nc.scalar.activation(sgi[:, :], gpB[:, 0:256], SIG)
                nc.scalar.activation(sgo[:, :], gpB[:, 256:512], SIG)
                nc.vector.tensor_mul(t1[:, :], sgi[:, :], tg[:, :])
                nc.vector.tensor_add(cT[h][:, :], t2[:, :], t1[:, :])
                nc.scalar.activation(tch[:, :], cT[h][:, :], TANH)
                nc.vector.tensor_mul(hT[cu][h][:, :], sgo[:, :], tch[:, :])

        for t in range(T):
            emit_half(t, 0)
            emit_half(t, 1)

        # final y tails
        for h in range(2):
            bsl = slice(h * BH, (h + 1) * BH)
            for k in range(HC):
                nc.tensor.matmul(
                    yp[h][:, :],
                    lhsT=wout[:, k * 128:(k + 1) * 128],
                    rhs=hT[(T - 1) % 2][h][:, k * 64:(k + 1) * 64],
                    start=False, stop=(k == HC - 1), skip_group_check=True,
                )
            tp = T - 1
            sl = ysbT[:, tp * B + h * BH:tp * B + (h + 1) * BH]
            nc.vector.scalar_tensor_tensor(
                sl, yp[h][0:YD, :], bscan[:, tp:tp + 1],
                y0T[:, bsl], op0=mybir.AluOpType.add, op1=mybir.AluOpType.add)
        lo = ((T - 1) // DMA_CHUNK) * DMA_CHUNK * B
        nc.sync.dma_start(d_out[:, lo:T * B], ysbT[:, lo:T * B])

    nc.compile()
    return nc


def kernel(z, x, W_ih, W_hh, b_ih, b_hh, W_proj, b_proj, W_out, b_out, y_pred_len):
    global LAST_RESULTS
    z = np.asarray(z, dtype=np.float32)
    x = np.asarray(x, dtype=np.float32)
    T = int(np.asarray(y_pred_len))

    consts = _prep_consts(
        np.asarray(W_ih, np.float32), np.asarray(W_hh, np.float32),
        np.asarray(b_ih, np.float32), np.asarray(b_hh, np.float32),
        np.asarray(W_proj, np.float32), np.asarray(b_proj, np.float32),
        np.asarray(W_out, np.float32), np.asarray(b_out, np.float32),
    )

    if T not in _BUILD_CACHE:
        _BUILD_CACHE[T] = _build(T)
    nc = _BUILD_CACHE[T]
    consts["bscan"] = np.ascontiguousarray(
        np.outer(np.asarray(b_out, np.float32),
                 np.arange(1, T + 1, dtype=np.float32)))

    bf = ml_dtypes.bfloat16
    in_maps = []
    for i in range(N_CORES):
        sl = slice(i * B, (i + 1) * B)
        m = dict(consts)
        m["zT"] = np.ascontiguousarray(z[sl].T.astype(bf))
        m["y0T"] = np.ascontiguousarray(x[sl, -1, :].T.astype(np.float32))
        in_maps.append(m)

    trace = bool(int(os.environ.get("BASS_KERNEL_TRACE", "0")))
    res = run_bass_kernel_spmd(
        nc, in_maps, core_ids=list(range(N_CORES)), trace=trace,
    )
    LAST_RESULTS = res

    outs = [np.ascontiguousarray(
                np.asarray(res.results[i]["out"]).reshape(YD, T, B).transpose(2, 1, 0))
            for i in range(N_CORES)]
    return np.concatenate(outs, axis=0)



# revision 29
# speedup vs baseline: 1.0038x; 1.0036x over previous
"""Trainium2 Bass kernel for nn_Decoder (LSTM decoder with residual output feedback).

Model (per batch row):
    h0 = c0 = z @ W_proj.T + b_proj                      # [B, H]
    y0 = x[:, -1, :]                                     # [B, X]
    per step t: gates = y_{t-1} @ W_ih.T + h_{t-1} @ W_hh.T + (b_ih + b_hh)
                i, f, g, o = split(gates); c = sig(f)*c + sig(i)*tanh(g)
                h = sig(o)*tanh(c); y_t = y_{t-1} + h @ W_out.T + b_out
    out = stack(y_1..y_T)                                # [B, T, Y]

Strategy:
  * Pure data-parallel over batch: B=1024 -> 128 rows/core on 8 NeuronCores,
    weights replicated, zero collectives; outputs concatenated on the host.
  * All state is kept TRANSPOSED on chip ([feature, batch], batch on the free
    axis): gates come out of the PE array as gates^T with the weights as the
    stationary operand, and the elementwise state update directly produces
    h^T, which feeds the next step's matmuls as the moving operand -- the
    recurrence contains no transposes at all.
  * Each core's 128 rows are further split into two independent 64-row halves
    executed half-a-step out of phase: while one half runs its sigmod/tanh +
    c/h update chain on the Scalar/Vector engines, the other half's matmul
    burst keeps the TensorEngine busy (and the PE clock un-throttled).
  * Matmuls run in bf16 (f32 PSUM accumulation); c and y state stay f32.
  * Gate rows are pre-permuted on the host gate-major (f,g | i,o) into two
    one-bank PSUM tiles: the f/g bank closes ~1.1us before each burst ends
    (so sig(f)/tanh(g)/f*c run hidden under the matmuls) and the i/o bank
    closes last, leaving only sig(i)->i*g->c+=->tanh(c)->o*tanh(c) on the
    recurrence-critical tail (all bf16, DVE 2x mode).
  * y_t accumulates in a persistent PSUM bank (y_t = y0 + psum); the output
    is staged transposed in SBUF and un-transposed on the host.
"""



import os
from contextlib import ExitStack

import ml_dtypes
import numpy as np

import concourse.bass as bass
import concourse.tile as tile
from concourse import bacc, mybir
from concourse.bass_utils import run_bass_kernel_spmd


F32 = mybir.dt.float32
BF16 = mybir.dt.bfloat16
SIG = mybir.ActivationFunctionType.Sigmoid
TANH = mybir.ActivationFunctionType.Tanh

B_TOT = 1024
N_CORES = 8
B = 128          # rows per core
BH = 64          # rows per half
ZD, XD, YD, H = 128, 64, 64, 512
HC, GC = 4, 16

LAST_RESULTS = None
_BUILD_CACHE = {}



def _gate_row_order():
    """Permuted gate-row order: gate-major blocks (f_0..f_3, g_0..g_3, i_0..i_3, o_0..o_3).

    Original gate layout along 4H: i=[0,512), f=[512,1024), g=[1024,1536), o=[1536,2048).
    Gate-major ordering makes every activation/elementwise op a CONTIGUOUS
    [128, 256] slice of the gate PSUM. The f/g gates go to PSUM bank A
    (closed ~1.1us before the burst ends, so sig(f)/tanh(g) run mid-burst)
    and i/o to bank B (whose residual chain after the burst is shortest:
    sig(i) -> i*g -> c+= -> tanh(c) -> o*tanh(c)).
    """
    idx = []
    for base in (512, 1024, 0, 1536):  # f, g, i, o
        for k in range(HC):
            idx.extend(range(base + k * 128, base + (k + 1) * 128))
    return np.asarray(idx)



def _prep_consts(W_ih, W_hh, b_ih, b_hh, W_proj, b_proj, W_out, b_out):
    bf = ml_dtypes.bfloat16
    order = _gate_row_order()
    Wg = W_hh[order]                       # [2048, 512] permuted rows
    Wi = W_ih[order]                       # [2048, 64]
    bt = (b_ih + b_hh)[order]              # [2048]

    wg_h = np.empty((128, HC * GC * 128), dtype=bf)
    for k in range(HC):
        for s in range(GC):
            blk = Wg[s * 128:(s + 1) * 128, k * 128:(k + 1) * 128].T  # [K,M]
            wg_h[:, (k * GC + s) * 128:(k * GC + s + 1) * 128] = blk.astype(bf)

    wg_y = np.empty((YD + 1, GC * 128), dtype=bf)
    for s in range(GC):
        wg_y[0:YD, s * 128:(s + 1) * 128] = Wi[s * 128:(s + 1) * 128, :].T.astype(bf)
        wg_y[YD, s * 128:(s + 1) * 128] = bt[s * 128:(s + 1) * 128].astype(bf)

    wout = np.zeros((128, HC * 128), dtype=bf)
    for k in range(HC):
        wout[:, k * 128:k * 128 + YD] = W_out[:, k * 128:(k + 1) * 128].T.astype(bf)

    wproj = np.empty((ZD, H), dtype=bf)
    for m in range(HC):
        wproj[:, m * 128:(m + 1) * 128] = W_proj[m * 128:(m + 1) * 128, :].T.astype(bf)

    bprojT = b_proj.reshape(HC, 128).T.copy().astype(np.float32)  # [128, HC]
    bout1 = b_out.reshape(1, YD).astype(bf)
    ones1 = np.ones((1, B), dtype=bf)
    return dict(wg_h=wg_h, wg_y=wg_y, wout=wout, wproj=wproj,
                bprojT=bprojT, bout1=bout1, ones1=ones1)



def _j4(ap, c):
    return ap.rearrange("p (j c) -> p j c", c=c)


def _build(T):
    nc = bacc.Bacc("TRN2", target_bir_lowering=False, debug=False)

    d_zT = nc.dram_tensor("zT", [ZD, B], BF16, kind="ExternalInput")
    d_y0T = nc.dram_tensor("y0T", [YD, B], F32, kind="ExternalInput")
    d_wg_h = nc.dram_tensor("wg_h", [128, HC * GC * 128], BF16, kind="ExternalInput")
    d_wg_y = nc.dram_tensor("wg_y", [YD + 1, GC * 128], BF16, kind="ExternalInput")
    d_wout = nc.dram_tensor("wout", [128, HC * 128], BF16, kind="ExternalInput")
    d_bout1 = nc.dram_tensor("bout1", [1, YD], BF16, kind="ExternalInput")
    d_ones1 = nc.dram_tensor("ones1", [1, B], BF16, kind="ExternalInput")
    d_wproj = nc.dram_tensor("wproj", [ZD, H], BF16, kind="ExternalInput")
    d_bprojT = nc.dram_tensor("bprojT", [128, HC], F32, kind="ExternalInput")
    d_bscan = nc.dram_tensor("bscan", [YD, T], F32, kind="ExternalInput")
    d_out = nc.dram_tensor("out", [YD, T * B], F32, kind="ExternalOutput")

    with ExitStack() as ctx:
        tc = ctx.enter_context(tile.TileContext(nc))
        const = ctx.enter_context(tc.tile_pool(name="const", bufs=1))
        state = ctx.enter_context(tc.tile_pool(name="state", bufs=1))
        actp = ctx.enter_context(tc.tile_pool(name="actp", bufs=4))
        gpsum = ctx.enter_context(tc.tile_pool(name="gpsum", bufs=3, space="PSUM"))
        ypsum = ctx.enter_context(tc.tile_pool(name="ypsum", bufs=1, space="PSUM"))

        wg_h = const.tile([128, HC * GC * 128], BF16)
        wg_y = const.tile([YD + 1, GC * 128], BF16)
        wout = const.tile([128, HC * 128], BF16)
        bout1 = const.tile([1, YD], BF16)
        ones1 = const.tile([1, B], BF16)
        wproj = const.tile([ZD, H], BF16)
        bprojT = const.tile([128, HC], F32)
        bscan = const.tile([YD, T], F32)
        zT = const.tile([ZD, B], BF16)
        y0T = const.tile([YD, B], F32)
        # small init-critical tensors first so h0/c0 compute (and the
        # one-time ACT table load) overlap the big wg_h weight DMA
        for sb, dr in (
            (zT, d_zT), (wproj, d_wproj), (bprojT, d_bprojT), (y0T, d_y0T),
            (ones1, d_ones1), (bout1, d_bout1), (bscan, d_bscan),
            (wg_y, d_wg_y), (wout, d_wout), (wg_h, d_wg_h),
        ):
            nc.sync.dma_start(sb[:, :], dr[:, :])

        # dummy activation to pull the ~2.7us ACT spline-table load off the
        # first decode step's critical path
        warmp = state.tile([1, YD], F32, name="actwarm")
        nc.scalar.activation(warmp[:, :], bout1[:, :], SIG)

        # per-half state; layout [128, 4*64]: H-chunk k at cols k*64
        cT = [state.tile([128, 256], BF16, name=f"cT{h}") for h in range(2)]
        hT = [[state.tile([128, 256], BF16, name=f"hT{p}_{h}") for h in range(2)]
              for p in range(2)]
        yTa = [[state.tile([YD + 1, BH], BF16, name=f"yTa{p}_{h}") for h in range(2)]
               for p in range(2)]
        ysbT = state.tile([YD, T * B], F32)   # out[y, t*128 + h*64 + b]
        yp = [ypsum.tile([128, BH], F32, name=f"yp{h}", tag=f"yp{h}")
              for h in range(2)]

        # --- init (both halves) ---
        for h in range(2):
            bsl = slice(h * BH, (h + 1) * BH)
            h0p = gpsum.tile([128, 512], F32, tag="gA", name=f"h0p{h}")
            for m in range(HC):
                nc.tensor.matmul(
                    h0p[:, m * 64:(m + 1) * 64],
                    lhsT=wproj[:, m * 128:(m + 1) * 128],
                    rhs=zT[:, bsl], start=True, stop=True,
                )
            for m in range(HC):
                nc.vector.tensor_scalar_add(
                    cT[h][:, m * 64:(m + 1) * 64],
                    h0p[:, m * 64:(m + 1) * 64],
                    bprojT[:, m:m + 1],
                )
            nc.vector.tensor_copy(hT[1][h][:, :], cT[h][:, :])
            nc.vector.tensor_copy(yTa[1][h][0:YD, :], y0T[:, bsl])
            nc.vector.memset(yTa[0][h][YD:YD + 1, :], 1.0)
            nc.vector.memset(yTa[1][h][YD:YD + 1, :], 1.0)

        DMA_CHUNK = 32

        def emit_half(t, h):
            pv = (t + 1) % 2
            cu = t % 2
            bsl = slice(h * BH, (h + 1) * BH)
            # two one-bank PSUM tiles: gpA = f,g gates, gpB = i,o gates.
            # Separate tiles -> dependency tracking frees bank A for the
            # activation chain while bank B is still accumulating.
            gpA = gpsum.tile([128, 512], F32, tag="gA", name=f"gpA{t}_{h}")
            gpB = gpsum.tile([128, 512], F32, tag="gB", name=f"gpB{t}_{h}")

            if t > 0:
                for k in range(HC):
                    nc.tensor.matmul(
                        yp[h][:, :],
                        lhsT=wout[:, k * 128:(k + 1) * 128],
                        rhs=hT[pv][h][:, k * 64:(k + 1) * 64],
                        start=(t == 1 and k == 0), stop=False,
                        skip_group_check=True,
                    )
                tp = t - 1
                # y_tp = y0 + psum + (tp+1)*b_out; the bias ramp comes from a
                # host-precomputed table so no bias matmul sits on this path.
                # The yTa write feeds THIS burst's wg_y matmuls below, so it
                # must be emitted before them.
                nc.vector.scalar_tensor_tensor(
                    yTa[tp % 2][h][0:YD, :], yp[h][0:YD, :], bscan[:, tp:tp + 1],
                    y0T[:, bsl], op0=mybir.AluOpType.add, op1=mybir.AluOpType.add)
                nc.vector.scalar_tensor_tensor(
                    ysbT[:, tp * B + h * BH:tp * B + (h + 1) * BH],
                    yp[h][0:YD, :], bscan[:, tp:tp + 1],
                    y0T[:, bsl], op0=mybir.AluOpType.add, op1=mybir.AluOpType.add)
                if h == 1 and (tp % DMA_CHUNK == DMA_CHUNK - 1):
                    lo = (tp // DMA_CHUNK) * DMA_CHUNK * B
                    nc.sync.dma_start(d_out[:, lo:(tp + 1) * B],
                                      ysbT[:, lo:(tp + 1) * B])

            def gates_for(gp, s0, ns):
                for s in range(s0, s0 + ns):
                    for k in range(HC):
                        nc.tensor.matmul(
                            gp[:, (s - s0) * 64:(s - s0 + 1) * 64],
                            lhsT=wg_h[:, (k * GC + s) * 128:(k * GC + s + 1) * 128],
                            rhs=hT[pv][h][:, k * 64:(k + 1) * 64],
                            start=(s == s0 and k == 0), stop=False,
                            skip_group_check=True,
                        )
                for s in range(s0, s0 + ns):
                    nc.tensor.matmul(
                        gp[:, (s - s0) * 64:(s - s0 + 1) * 64],
                        lhsT=wg_y[:, s * 128:(s + 1) * 128],
                        rhs=yTa[pv][h][:, :],
                        start=False, stop=True, skip_group_check=True,
                    )

            gates_for(gpA, 0, 8)   # f, g gates: bank A closes ~1.1us before burst end
            gates_for(gpB, 8, 8)   # i, o gates: bank B closes at burst end

            # chain: sig(f)/tanh(g)/f*c run mid-burst off bank A; the tail
            # after the burst is only sig(i) -> i*g -> c+= -> tanh(c) -> h,
            # with sig(o) filling the ACT engine's idle slot between sig(i)
            # and tanh(c) so the greedy scheduler doesn't park a slack op of
            # the next half-step there (that costs ~0.5us on the PE).
            # bf16 intermediates put every tail mul in the DVE 2x perf mode.
            sgf = actp.tile([128, 256], BF16, tag=f"sgf{h}", name=f"sgf{t}_{h}")
            tg = actp.tile([128, 256], BF16, tag=f"tg{h}", name=f"tg{t}_{h}")
            sgi = actp.tile([128, 256], BF16, tag=f"sgi{h}", name=f"sgi{t}_{h}")
            sgo = actp.tile([128, 256], BF16, tag=f"sgo{h}", name=f"sgo{t}_{h}")
            t2 = actp.tile([128, 256], BF16, tag=f"t2{h}", name=f"t2_{t}_{h}")
            t1 = actp.tile([128, 256], BF16, tag=f"t1{h}", name=f"t1_{t}_{h}")
            nc.scalar.activation(sgf[:, :], gpA[:, 0:256], SIG)
            nc.vector.tensor_mul(t2[:, :], sgf[:, :], cT[h][:, :])
            nc.scalar.activation(tg[:, :], gpA[:, 256:512], TANH)
            tch = actp.tile([128, 256], BF16, tag=f"tc{h}", name=f"tc{t}_{h}")
            with tc.high_priority():
                nc.scalar.activation(sgi[:, :], gpB[:, 0:256], SIG)
                nc.scalar.activation(sgo[:, :], gpB[:, 256:512], SIG)
                nc.vector.tensor_mul(t1[:, :], sgi[:, :], tg[:, :])
                nc.vector.tensor_add(cT[h][:, :], t2[:, :], t1[:, :])
                nc.scalar.activation(tch[:, :], cT[h][:, :], TANH)
                nc.vector.tensor_mul(hT[cu][h][:, :], sgo[:, :], tch[:, :])

        for t in range(T):
            emit_half(t, 0)
            emit_half(t, 1)

        # final y tails
        for h in range(2):
            bsl = slice(h * BH, (h + 1) * BH)
            for k in range(HC):
                nc.tensor.matmul(
                    yp[h][:, :],
                    lhsT=wout[:, k * 128:(k + 1) * 128],
                    rhs=hT[(T - 1) % 2][h][:, k * 64:(k + 1) * 64],
                    start=False, stop=(k == HC - 1), skip_group_check=True,
                )
            tp = T - 1
            sl = ysbT[:, tp * B + h * BH:tp * B + (h + 1) * BH]
            nc.vector.scalar_tensor_tensor(
                sl, yp[h][0:YD, :], bscan[:, tp:tp + 1],
                y0T[:, bsl], op0=mybir.AluOpType.add, op1=mybir.AluOpType.add)
        lo = ((T - 1) // DMA_CHUNK) * DMA_CHUNK * B
        nc.sync.dma_start(d_out[:, lo:T * B], ysbT[:, lo:T * B])

    nc.compile()
    return nc


def kernel(z, x, W_ih, W_hh, b_ih, b_hh, W_proj, b_proj, W_out, b_out, y_pred_len):
    global LAST_RESULTS
    z = np.asarray(z, dtype=np.float32)
    x = np.asarray(x, dtype=np.float32)
    T = int(np.asarray(y_pred_len))

    consts = _prep_consts(
        np.asarray(W_ih, np.float32), np.asarray(W_hh, np.float32),
        np.asarray(b_ih, np.float32), np.asarray(b_hh, np.float32),
        np.asarray(W_proj, np.float32), np.asarray(b_proj, np.float32),
        np.asarray(W_out, np.float32), np.asarray(b_out, np.float32),
    )

    if T not in _BUILD_CACHE:
        _BUILD_CACHE[T] = _build(T)
    nc = _BUILD_CACHE[T]
    consts["bscan"] = np.ascontiguousarray(
        np.outer(np.asarray(b_out, np.float32),
                 np.arange(1, T + 1, dtype=np.float32)))

    bf = ml_dtypes.bfloat16
    in_maps = []
    for i in range(N_CORES):
        sl = slice(i * B, (i + 1) * B)
        m = dict(consts)
        m["zT"] = np.ascontiguousarray(z[sl].T.astype(bf))
        m["y0T"] = np.ascontiguousarray(x[sl, -1, :].T.astype(np.float32))
        in_maps.append(m)

    trace = bool(int(os.environ.get("BASS_KERNEL_TRACE", "0")))
    res = run_bass_kernel_spmd(
        nc, in_maps, core_ids=list(range(N_CORES)), trace=trace,
    )
    LAST_RESULTS = res

    outs = [np.ascontiguousarray(
                np.asarray(res.results[i]["out"]).reshape(YD, T, B).transpose(2, 1, 0))
            for i in range(N_CORES)]
    return np.concatenate(outs, axis=0)

